# revision 4
# baseline (speedup 1.0000x reference)
"""CCLoss (Pearson correlation loss) Trainium2 kernel, 8-way data parallel.

Problem: y_pred ~ (64,1,480,640) f32, y_true ~ (64,1,480,640) f32.
reference: per-sample z-score (ddof=1) over (1,480,640), r = corr-like ratio,
loss = -mean(r).

Strategy: shard batch (64) across 8 cores, 8 samples/core. Each core computes
per-sample, per-partition moment partials in a single pass over the data
(memory-bound regime):
  - ScalarE (ACT):  sum(y^2) and sum(y) via activation accum_out (two passes)
  - VectorE (DVE):  sum(x*y) via tensor_tensor_reduce; mean(x)/var(x) via
                    bn_stats/bn_aggr
Partition-axis reduction and the final scalar math run on host in float64.
"""
import os
import sys

import numpy as np

for _p in ("/opt/trn_rl_repo", "/root/.axon_site/_ro/trn_rl_repo"):
    if os.path.isdir(_p) and _p not in sys.path:
        sys.path.append(_p)

import concourse.bass as bass
import concourse.mybir as mybir
import concourse.tile as tile
from concourse import bacc
from concourse.bass_utils import run_bass_kernel_spmd

NCORES = 8
B = 64
SPB = B // NCORES          # samples per core
P = 128                    # SBUF partitions
N = 1 * 480 * 640          # elements per sample
F = N // P                 # free dim per partition (2400)
NCHUNK = 5                 # bn_stats hardware limit: <=512 elems per call
CW = F // NCHUNK           # 480
EPS = 1e-8

FP32 = mybir.dt.float32

_CACHE = {}
LAST_RESULTS = None


def _build():
    nc = bacc.Bacc("TRN2", target_bir_lowering=False, debug=False,
                   enable_asserts=False)
    yp_d = nc.dram_tensor("yp", (SPB, P, F), FP32, kind="ExternalInput").ap()
    yt_d = nc.dram_tensor("yt", (SPB, P, F), FP32, kind="ExternalInput").ap()
    # per-partition partials, gathered on host
    sx_d = nc.dram_tensor("sx", (P, 2 * SPB), FP32, kind="ExternalOutput").ap()
    syy_d = nc.dram_tensor("syy", (P, SPB), FP32, kind="ExternalOutput").ap()
    sy_d = nc.dram_tensor("sy", (P, SPB), FP32, kind="ExternalOutput").ap()
    sxy_d = nc.dram_tensor("sxy", (P, SPB), FP32, kind="ExternalOutput").ap()

    with tile.TileContext(nc) as tc:
        with (
            tc.tile_pool(name="data", bufs=3) as data,
            tc.tile_pool(name="scratch", bufs=2) as scratch,
            tc.tile_pool(name="stats", bufs=2) as stats,
            tc.tile_pool(name="persist", bufs=1) as persist,
        ):
            st_x = persist.tile([P, 2 * SPB], FP32)   # (mean, var) per sample
            st_yy = persist.tile([P, SPB], FP32)
            st_y = persist.tile([P, SPB], FP32)
            st_xy = persist.tile([P, SPB], FP32)
            nc.vector.memset(st_x[:], 0.0)
            nc.vector.memset(st_xy[:], 0.0)
            nc.vector.memset(st_yy[:], 0.0)
            nc.vector.memset(st_y[:], 0.0)

            for s in range(SPB):
                xt = data.tile([P, F], FP32)
                nc.sync.dma_start(xt[:], yp_d[s])
                yt = data.tile([P, F], FP32)
                nc.sync.dma_start(yt[:], yt_d[s])

                # ScalarE: sum(y^2), sum(y)
                sq = scratch.tile([P, F], FP32)
                nc.scalar.activation(
                    sq[:], yt[:], mybir.ActivationFunctionType.Square,
                    accum_out=st_yy[:, s:s + 1],
                )
                cpy = scratch.tile([P, F], FP32)
                nc.scalar.activation(
                    cpy[:], yt[:], mybir.ActivationFunctionType.Copy,
                    accum_out=st_y[:, s:s + 1],
                )

                # VectorE: sum(x*y)  (tensor_tensor_reduce crashes TRN2 here,
                # scalar_tensor_tensor's accum path is the working equivalent)
                prod = scratch.tile([P, F], FP32)
                nc.vector.scalar_tensor_tensor(
                    out=prod[:], in0=xt[:], scalar=1.0, in1=yt[:],
                    op0=mybir.AluOpType.mult, op1=mybir.AluOpType.mult,
                    accum_out=st_xy[:, s:s + 1],
                )

                # VectorE: mean/var of x per partition
                st6 = stats.tile([P, NCHUNK, 6], FP32)
                for c in range(NCHUNK):
                    nc.vector.bn_stats(st6[:, c, :], xt[:, c * CW:(c + 1) * CW])
                nc.vector.bn_aggr(st_x[:, 2 * s:2 * s + 2], st6[:])

            nc.sync.dma_start(sx_d[:], st_x[:])
            nc.sync.dma_start(syy_d[:], st_yy[:])
            nc.sync.dma_start(sy_d[:], st_y[:])
            nc.sync.dma_start(sxy_d[:], st_xy[:])

    nc.compile()
    return nc


def _get_nc():
    if "nc" not in _CACHE:
        _CACHE["nc"] = _build()
    return _CACHE["nc"]


def kernel(y_pred: np.ndarray, y_true: np.ndarray) -> np.ndarray:
    global LAST_RESULTS
    nc = _get_nc()

    yp = np.ascontiguousarray(np.asarray(y_pred, dtype=np.float32).reshape(B, P, F))
    yt = np.ascontiguousarray(np.asarray(y_true, dtype=np.float32).reshape(B, P, F))

    in_maps = [
        {"yp": yp[c * SPB:(c + 1) * SPB], "yt": yt[c * SPB:(c + 1) * SPB]}
        for c in range(NCORES)
    ]
    trace = bool(os.environ.get("CCLOSS_TRACE"))
    try:
        res = run_bass_kernel_spmd(nc, in_maps, core_ids=list(range(NCORES)),
                                   trace=trace)
    except Exception:
        if not trace:
            raise
        res = run_bass_kernel_spmd(nc, in_maps, core_ids=list(range(NCORES)),
                                   trace=False)
    LAST_RESULTS = res

    r_all = np.empty(B, dtype=np.float64)
    n = float(N)
    for c in range(NCORES):
        out = res.results[c]
        sx = out["sx"].astype(np.float64)      # [P, 2*SPB] (mean, var)
        syy = out["syy"].astype(np.float64)    # [P, SPB]
        sy = out["sy"].astype(np.float64)
        sxy = out["sxy"].astype(np.float64)
        for s in range(SPB):
            mean_p = sx[:, 2 * s]
            var_p = sx[:, 2 * s + 1]
            Sx = F * mean_p.sum()
            Sxx = F * (var_p + mean_p * mean_p).sum()
            Syy = syy[:, s].sum()
            Sy = sy[:, s].sum()
            Sxy = sxy[:, s].sum()

            cxx = Sxx - Sx * Sx / n            # sum((x-mu_x)^2)
            cyy = Syy - Sy * Sy / n
            cxy = Sxy - Sx * Sy / n
            sdx = np.sqrt(cxx / (n - 1.0)) + EPS
            sdy = np.sqrt(cyy / (n - 1.0)) + EPS

            num = cxy / (sdx * sdy)            # sum(a*b)
            saa = cxx / (sdx * sdx)            # sum(a*a)
            sbb = cyy / (sdy * sdy)
            r = num / np.sqrt(saa * sbb + EPS)
            r_all[c * SPB + s] = r

    loss = -r_all.mean()
    return np.array(loss, dtype=np.float32)


# revision 6
# speedup vs baseline: 1.1152x; 1.1152x over previous
"""CCLoss (Pearson correlation loss) Trainium2 kernel, 8-way data parallel.

Problem: y_pred ~ (64,1,480,640) f32, y_true ~ (64,1,480,640) f32.
reference: per-sample z-score (ddof=1) over (1,480,640), r = corr-like ratio,
loss = -mean(r).

Strategy: shard batch (64) across 8 cores, 8 samples/core. Each core computes
per-sample, per-partition moment partials in a single pass over the data
(memory-bound regime):
  - ScalarE (ACT):  sum(y^2) and sum(y) via activation accum_out (two passes)
  - VectorE (DVE):  sum(x*y) via tensor_tensor_reduce; mean(x)/var(x) via
                    bn_stats/bn_aggr
Partition-axis reduction and the final scalar math run on host in float64.
"""
import os
import sys

import numpy as np

for _p in ("/opt/trn_rl_repo", "/root/.axon_site/_ro/trn_rl_repo"):
    if os.path.isdir(_p) and _p not in sys.path:
        sys.path.append(_p)

import concourse.bass as bass
import concourse.mybir as mybir
import concourse.tile as tile
from concourse import bacc
from concourse.bass_utils import run_bass_kernel_spmd

NCORES = 8
B = 64
SPB = B // NCORES          # samples per core
P = 128                    # SBUF partitions
N = 1 * 480 * 640          # elements per sample
F = N // P                 # free dim per partition (2400)
NCHUNK = 5                 # bn_stats hardware limit: <=512 elems per call
CW = F // NCHUNK           # 480
EPS = 1e-8

FP32 = mybir.dt.float32

_CACHE = {}
LAST_RESULTS = None


def _build():
    nc = bacc.Bacc("TRN2", target_bir_lowering=False, debug=False,
                   enable_asserts=False)
    yp_d = nc.dram_tensor("yp", (SPB, P, F), FP32, kind="ExternalInput").ap()
    yt_d = nc.dram_tensor("yt", (SPB, P, F), FP32, kind="ExternalInput").ap()
    # per-partition partials, gathered on host
    sx_d = nc.dram_tensor("sx", (P, 2 * SPB), FP32, kind="ExternalOutput").ap()
    syy_d = nc.dram_tensor("syy", (P, SPB), FP32, kind="ExternalOutput").ap()
    sy_d = nc.dram_tensor("sy", (P, SPB), FP32, kind="ExternalOutput").ap()
    sxy_d = nc.dram_tensor("sxy", (P, SPB), FP32, kind="ExternalOutput").ap()

    with tile.TileContext(nc) as tc:
        with (
            tc.tile_pool(name="data", bufs=6) as data,
            tc.tile_pool(name="scratch", bufs=4) as scratch,
            tc.tile_pool(name="stats", bufs=2) as stats,
            tc.tile_pool(name="persist", bufs=1) as persist,
        ):
            st_x = persist.tile([P, 2 * SPB], FP32)   # (mean, var) per sample
            st_yy = persist.tile([P, SPB], FP32)
            st_y = persist.tile([P, SPB], FP32)
            st_xy = persist.tile([P, SPB], FP32)
            nc.vector.memset(st_x[:], 0.0)
            nc.vector.memset(st_xy[:], 0.0)
            nc.vector.memset(st_yy[:], 0.0)
            nc.vector.memset(st_y[:], 0.0)

            for s in range(SPB):
                xt = data.tile([P, F], FP32)
                nc.sync.dma_start(xt[:], yp_d[s])
                yt = data.tile([P, F], FP32)
                nc.sync.dma_start(yt[:], yt_d[s])

                # ScalarE: sum(y^2), sum(y)
                sq = scratch.tile([P, F], FP32, tag="junk")
                nc.scalar.activation(
                    sq[:], yt[:], mybir.ActivationFunctionType.Square,
                    accum_out=st_yy[:, s:s + 1],
                )
                cpy = scratch.tile([P, F], FP32, tag="junk")
                nc.scalar.activation(
                    cpy[:], yt[:], mybir.ActivationFunctionType.Copy,
                    accum_out=st_y[:, s:s + 1],
                )

                # VectorE: sum(x*y)  (tensor_tensor_reduce crashes TRN2 here,
                # scalar_tensor_tensor's accum path is the working equivalent)
                prod = scratch.tile([P, F], FP32, tag="junk")
                nc.vector.scalar_tensor_tensor(
                    out=prod[:], in0=xt[:], scalar=1.0, in1=yt[:],
                    op0=mybir.AluOpType.mult, op1=mybir.AluOpType.mult,
                    accum_out=st_xy[:, s:s + 1],
                )

                # VectorE: mean/var of x per partition
                st6 = stats.tile([P, NCHUNK, 6], FP32)
                for c in range(NCHUNK):
                    nc.vector.bn_stats(st6[:, c, :], xt[:, c * CW:(c + 1) * CW])
                nc.vector.bn_aggr(st_x[:, 2 * s:2 * s + 2], st6[:])

            nc.sync.dma_start(sx_d[:], st_x[:])
            nc.sync.dma_start(syy_d[:], st_yy[:])
            nc.sync.dma_start(sy_d[:], st_y[:])
            nc.sync.dma_start(sxy_d[:], st_xy[:])

    nc.compile()
    return nc


def _get_nc():
    if "nc" not in _CACHE:
        _CACHE["nc"] = _build()
    return _CACHE["nc"]


def kernel(y_pred: np.ndarray, y_true: np.ndarray) -> np.ndarray:
    global LAST_RESULTS
    nc = _get_nc()

    yp = np.ascontiguousarray(np.asarray(y_pred, dtype=np.float32).reshape(B, P, F))
    yt = np.ascontiguousarray(np.asarray(y_true, dtype=np.float32).reshape(B, P, F))

    in_maps = [
        {"yp": yp[c * SPB:(c + 1) * SPB], "yt": yt[c * SPB:(c + 1) * SPB]}
        for c in range(NCORES)
    ]
    trace = bool(os.environ.get("CCLOSS_TRACE"))
    try:
        res = run_bass_kernel_spmd(nc, in_maps, core_ids=list(range(NCORES)),
                                   trace=trace)
    except Exception:
        if not trace:
            raise
        res = run_bass_kernel_spmd(nc, in_maps, core_ids=list(range(NCORES)),
                                   trace=False)
    LAST_RESULTS = res

    r_all = np.empty(B, dtype=np.float64)
    n = float(N)
    for c in range(NCORES):
        out = res.results[c]
        sx = out["sx"].astype(np.float64)      # [P, 2*SPB] (mean, var)
        syy = out["syy"].astype(np.float64)    # [P, SPB]
        sy = out["sy"].astype(np.float64)
        sxy = out["sxy"].astype(np.float64)
        for s in range(SPB):
            mean_p = sx[:, 2 * s]
            var_p = sx[:, 2 * s + 1]
            Sx = F * mean_p.sum()
            Sxx = F * (var_p + mean_p * mean_p).sum()
            Syy = syy[:, s].sum()
            Sy = sy[:, s].sum()
            Sxy = sxy[:, s].sum()

            cxx = Sxx - Sx * Sx / n            # sum((x-mu_x)^2)
            cyy = Syy - Sy * Sy / n
            cxy = Sxy - Sx * Sy / n
            sdx = np.sqrt(cxx / (n - 1.0)) + EPS
            sdy = np.sqrt(cyy / (n - 1.0)) + EPS

            num = cxy / (sdx * sdy)            # sum(a*b)
            saa = cxx / (sdx * sdx)            # sum(a*a)
            sbb = cyy / (sdy * sdy)
            r = num / np.sqrt(saa * sbb + EPS)
            r_all[c * SPB + s] = r

    loss = -r_all.mean()
    return np.array(loss, dtype=np.float32)
